# revision 1
# baseline (speedup 1.0000x reference)
"""Char-LSTM kernel for Trainium2 (8 NeuronCores, data parallel).

Strategy
--------
Host side:
  * Precompute G = emb @ W_ih.T + b_ih + b_hh  (vocab=100 -> [100, 4H]).
    The per-step embedding+input-projection then becomes a gather of G rows,
    which we realize on-device as an exact one-hot matmul accumulating
    directly into the same PSUM region as the recurrent matmul.
  * Sort words by length, deal them into per-core blocks of 512 words of a
    single length each (padded with dummies); leftovers go to "overflow"
    blocks which run all 16 steps with per-step h capture.
  * Blocks are paired into groups of 1024 words: block A lives on SBUF
    partitions 0:64, block B on 64:128 (state stored transposed, [H, words]).

Device side (identical SPMD program on all 8 cores):
  Per group-step:
    * 8 one-hot matmuls (vocab split 0:64 / 64:100 across PE row-groups) and
      8 recurrent matmuls (K=64), M=64 each, accumulating into one
      [128, 2048] PSUM tile laid out as banks [i | f | o | g] with block A in
      partitions 0:64 and block B in 64:128.
    * One sigmoid over [128, 1536] (i,f,o), tanh over g, then the cell update
      on the Vector engine, tanh(c) and h = o * tanh(c).
  Groups are emitted interleaved ~3 wide so the recurrence chains of
  independent groups pipeline across the Tensor/Scalar/Vector engines.
"""

import os
import sys

for _p in ("/opt/trn_rl_repo", "/root/.axon_site/_ro/trn_rl_repo"):
    if os.path.isdir(_p) and _p not in sys.path:
        sys.path.insert(0, _p)

import numpy as np
import ml_dtypes

BF16 = ml_dtypes.bfloat16

H = 64          # hidden size
E = 32          # char embedding size
V = 100         # vocab
MAXL = 16       # max word length
BLK = 512       # words per block (one half of a group)
NCORES = 8
GATE4 = 4 * H   # 256

# torch gate order in the weights is [i, f, g, o]; we stage banks as
# [i, f, o, g] so sigmoid covers one contiguous [128, 1536] span.
_GATE_PERM = np.concatenate([
    np.arange(0, 64),        # i
    np.arange(64, 128),      # f
    np.arange(192, 256),     # o
    np.arange(128, 192),     # g
])

INTERLEAVE = int(__import__("os").environ.get("LSTM_INTERLEAVE", "3"))
SKIPB = __import__("os").environ.get("LSTM_SKIPB", "1") == "1"
_PROGRAM_CACHE = {}


# --------------------------------------------------------------------------
# Host-side planning
# --------------------------------------------------------------------------

def _plan(lengths):
    """Assign words to (core, block, column) slots.

    Returns dict with:
      blocks: list (shared across cores) of dicts {L, is_ov, ov_idx}
      groups: list of dicts {a, b, steps} (block indices)
      sched:  emission order list of (group_idx, t)
      assign: per core: list of np arrays [BLK] of word ids (-1 = dummy),
              aligned with blocks
    """
    n = lengths.shape[0]
    lengths = lengths.astype(np.int64)
    order = np.argsort(lengths, kind="stable")

    per_core_words = [[] for _ in range(NCORES)]   # per core: list of [BLK] arrays
    block_meta = []                                # shared: (L, is_ov)

    leftovers = []
    for L in range(1, MAXL + 1):
        idx = order[np.searchsorted(lengths, L, side="left", sorter=order):
                    np.searchsorted(lengths, L, side="right", sorter=order)]
        take = idx[: NCORES * BLK]
        leftovers.append(idx[NCORES * BLK:])
        arr = np.full(NCORES * BLK, -1, dtype=np.int64)
        arr[: take.shape[0]] = take
        arr = arr.reshape(NCORES, BLK)
        for c in range(NCORES):
            per_core_words[c].append(arr[c])
        block_meta.append((L, False))

    leftovers = np.concatenate(leftovers) if leftovers else np.empty(0, np.int64)

    # Try to fold leftover words into the free slots of the length-16 block
    # (which then runs per-step capture); fall back to dedicated overflow
    # blocks when they don't fit.
    l16 = MAXL - 1  # index of the length-16 block in block_meta order
    free16 = [int((per_core_words[c][l16] < 0).sum()) for c in range(NCORES)]
    if leftovers.shape[0] <= sum(free16):
        block_meta[l16] = (MAXL, True)
        pos = 0
        for c in range(NCORES):
            k = min(free16[c], leftovers.shape[0] - pos)
            if k > 0:
                arr = per_core_words[c][l16]
                slots = np.nonzero(arr < 0)[0][:k]
                arr[slots] = leftovers[pos:pos + k]
                pos += k
        leftovers = leftovers[:0]

    if leftovers.shape[0]:
        n_ov = -(-leftovers.shape[0] // (NCORES * BLK))
        ov = np.full(n_ov * NCORES * BLK, -1, dtype=np.int64)
        ov[: leftovers.shape[0]] = leftovers
        ov = ov.reshape(n_ov, NCORES, BLK)
        for i in range(n_ov):
            for c in range(NCORES):
                per_core_words[c].append(ov[i, c])
            block_meta.append((MAXL, True))

    if len(block_meta) % 2 == 1:
        for c in range(NCORES):
            per_core_words[c].append(np.full(BLK, -1, dtype=np.int64))
        block_meta.append((1, False))

    # Sort blocks: descending length, overflow blocks first among equals so
    # they pair with the longest regular block.
    nb = len(block_meta)
    key = sorted(range(nb), key=lambda b: (-block_meta[b][0], not block_meta[b][1]))
    blocks = []
    ov_count = 0
    for b in key:
        L, is_ov = block_meta[b]
        blocks.append({"L": L, "is_ov": is_ov,
                       "ov_idx": (ov_count if is_ov else -1), "orig": b})
        if is_ov:
            ov_count += 1

    assign = [[per_core_words[c][blocks[i]["orig"]] for i in range(nb)]
              for c in range(NCORES)]

    groups = []
    for i in range(0, nb, 2):
        groups.append({"a": i, "b": i + 1,
                       "steps": max(blocks[i]["L"], blocks[i + 1]["L"])})

    # Greedy interleaved schedule: each round, one step of the (up to) 3
    # groups with the most remaining work.
    remaining = [g["steps"] for g in groups]
    next_t = [0] * len(groups)
    sched = []
    while any(r > 0 for r in remaining):
        act = sorted(range(len(groups)), key=lambda g: -remaining[g])[:INTERLEAVE]
        act = [g for g in act if remaining[g] > 0]
        for g in act:
            sched.append((g, next_t[g]))
            next_t[g] += 1
            remaining[g] -= 1

    # capture steps: for each capture block, the union (over cores) of
    # final steps of its words with length < MAXL, plus MAXL-1 (so length-16
    # words folded into a capture block are also covered).
    for bi, blk in enumerate(blocks):
        if not blk["is_ov"]:
            continue
        steps = set()
        for c in range(NCORES):
            w = assign[c][bi]
            w = w[w >= 0]
            steps.update((lengths[w] - 1).tolist())
        blk["cap_steps"] = tuple(sorted(steps))

    return {"blocks": blocks, "groups": groups, "sched": sched,
            "assign": assign, "n_ov": ov_count}


def _build_onehots(plan, chars, lengths):
    """Per-core one-hot slab tensors [n_slabs, V, BLK] float32.

    Slab order matches the device program's emission order: for each
    scheduled (group, t): A half then B half.
    """
    blocks, groups, sched = plan["blocks"], plan["groups"], plan["sched"]
    n_slabs = 2 * len(sched)
    out = []
    for c in range(NCORES):
        oh = np.zeros((n_slabs, V, BLK), dtype=BF16)
        slab = 0
        for (g, t) in sched:
            for blk_idx in (groups[g]["a"], groups[g]["b"]):
                words = plan["assign"][c][blk_idx]
                valid = (words >= 0)
                w = words[valid]
                if w.shape[0]:
                    alive = t < lengths[w]
                    cols = np.nonzero(valid)[0][alive]
                    ch = chars[w[alive], t]
                    oh[slab, ch, cols] = 1.0
                slab += 1
        out.append(oh)
    return out


# --------------------------------------------------------------------------
# Device program
# --------------------------------------------------------------------------

def _build_program(plan_sig, blocks, groups, sched, n_ov, variant="full",
                   reps=1):
    import concourse.bass as bass
    import concourse.tile as tile
    from concourse import bacc, mybir
    from contextlib import nullcontext

    do_mm = variant not in ("nomm", "onemm")
    one_mm = variant == "onemm"
    do_act = variant not in ("noact",)
    do_dma = variant not in ("nodma",)

    f32 = mybir.dt.float32
    bf16 = mybir.dt.bfloat16
    n_blocks = len(blocks)
    n_slabs = 2 * len(sched)

    nc = bacc.Bacc("TRN2", target_bir_lowering=False, debug=False,
                   num_devices=NCORES)
    oh_d = nc.dram_tensor("oh", [n_slabs, V, BLK], bf16, kind="ExternalInput")
    gtab_d = nc.dram_tensor("gtab", [128, GATE4], bf16, kind="ExternalInput")
    whha_d = nc.dram_tensor("whha", [128, GATE4], bf16, kind="ExternalInput")
    whhb_d = nc.dram_tensor("whhb", [128, GATE4], bf16, kind="ExternalInput")
    out_d = nc.dram_tensor("out", [n_blocks, H, BLK], f32, kind="ExternalOutput")
    ov_d = nc.dram_tensor("ov", [max(1, n_ov) * MAXL, H, BLK], f32,
                          kind="ExternalOutput")

    with tile.TileContext(nc) as tc:
        with (
            tc.tile_pool(name="consts", bufs=1) as consts,
            tc.tile_pool(name="slabs", bufs=10) as slabs,
            tc.tile_pool(name="psum", bufs=2, space="PSUM") as psump,
            tc.tile_pool(name="sig", bufs=4) as sigp,
            tc.tile_pool(name="gt", bufs=3) as gtp,
            tc.tile_pool(name="tc_", bufs=3) as tcp,
            tc.tile_pool(name="tmp", bufs=4) as tmpp,
            tc.tile_pool(name="state", bufs=8) as statep,
            tc.tile_pool(name="ovst", bufs=3) as ovstp,
        ):
            gtab = consts.tile([128, GATE4], bf16, tag="gtab")
            whha = consts.tile([128, GATE4], bf16, tag="whha")
            whhb = consts.tile([128, GATE4], bf16, tag="whhb")
            nc.sync.dma_start(out=gtab[:], in_=gtab_d[:])
            nc.sync.dma_start(out=whha[:], in_=whha_d[:])
            nc.sync.dma_start(out=whhb[:], in_=whhb_d[:])

            loop_cm = tc.For_i(0, reps, 1) if reps > 1 else nullcontext()
            with loop_cm:
                gstate = {}
                slab_idx = 0
                for (g, t) in sched:
                    grp = groups[g]
                    a, b = blocks[grp["a"]], blocks[grp["b"]]
                    La, Lb = a["L"], b["L"]

                    # gpsimd wants 32-aligned partition ranges: zero [96:128],
                    # the DMA then overwrites rows 96:100 with real one-hot data.
                    sA = slabs.tile([128, BLK], bf16, tag="slab", name="sA")
                    nc.gpsimd.memset(sA[96:128, :], 0.0)
                    if do_dma:
                        nc.sync.dma_start(out=sA[0:V, :], in_=oh_d[slab_idx])
                    slab_idx += 1
                    if t < Lb or not SKIPB:
                        sB = slabs.tile([128, BLK], bf16, tag="slab", name="sB")
                        nc.gpsimd.memset(sB[96:128, :], 0.0)
                        if do_dma:
                            nc.sync.dma_start(out=sB[0:V, :], in_=oh_d[slab_idx])
                    slab_idx += 1

                    ps = psump.tile([128, 4 * BLK], f32, tag="ps")
                    st = gstate.get(g)

                    # All matmuls use K=128 (vocab zero-padded; whhA/whhB have a
                    # zero half so block A/B recurrences pick out their own h).
                    # Per bank, A's accumulation group fully precedes B's: B's
                    # start=True clears the bank's has_written bits, which is only
                    # safe once A's group is complete.
                    if do_mm:
                        for q in range(4):
                            qs = slice(64 * q, 64 * q + 64)
                            cs = slice(BLK * q, BLK * q + BLK)
                            oA = ps[0:64, cs]
                            nc.tensor.matmul(oA, gtab[:, qs], sA[:, :],
                                             start=True, stop=(t == 0),
                                             tile_position=(0, 0))
                            if t > 0:
                                nc.tensor.matmul(oA, whha[:, qs], st["hb"][:, :],
                                                 start=False, stop=True,
                                                 tile_position=(0, 0))
                            if t < Lb or not SKIPB:
                                oB = ps[64:128, cs]
                                nc.tensor.matmul(oB, gtab[:, qs], sB[:, :],
                                                 start=True, stop=(t == 0),
                                                 tile_position=(0, 64))
                                if t > 0:
                                    nc.tensor.matmul(oB, whhb[:, qs],
                                                     st["hb"][:, :],
                                                     start=False, stop=True,
                                                     tile_position=(0, 64))

                    if one_mm:
                        nc.tensor.matmul(ps[0:64, 0:BLK], gtab[:, 0:64], sA[:, :],
                                         start=True, stop=True,
                                         tile_position=(0, 0))
                    if t == 0:
                        st = gstate[g] = {
                            "hb": statep.tile([128, BLK], bf16, tag="hb",
                                              name="hb"),
                            "c": statep.tile([128, BLK], f32, tag="c", name="c"),
                        }
                        if not do_act:
                            nc.gpsimd.memset(st["hb"][:, :], 0.0)
                            nc.gpsimd.memset(st["c"][:, :], 0.0)

                    cap_halves = [
                        (blk, half) for blk, half in
                        ((a, slice(0, 64)), (b, slice(64, 128)))
                        if blk["is_ov"] and t in blk.get("cap_steps", ())
                    ]
                    need_f32_h = (t == La - 1) or (t == Lb - 1) or bool(cap_halves)
                    # Once the shorter block B is finished, restrict the whole
                    # chain to A's partitions (same column cost, but avoids
                    # reading PSUM regions that were never written this step).
                    sl = slice(0, 128 if (t < Lb or not SKIPB) else 64)
                    if do_act:
                        sig = sigp.tile([128, 3 * BLK], f32, tag="sig")
                        nc.scalar.activation(out=sig[sl, :], in_=ps[sl, 0:3 * BLK],
                                             func=mybir.ActivationFunctionType.Sigmoid)
                        gt = gtp.tile([128, BLK], f32, tag="gt")
                        nc.scalar.activation(out=gt[sl, :], in_=ps[sl, 3 * BLK:4 * BLK],
                                             func=mybir.ActivationFunctionType.Tanh)

                        if t == 0:
                            nc.vector.tensor_mul(st["c"][sl, :], sig[sl, 0:BLK],
                                                 gt[sl, :])
                        else:
                            t1 = tmpp.tile([128, BLK], f32, tag="t1")
                            t2 = tmpp.tile([128, BLK], f32, tag="t2")
                            nc.vector.tensor_mul(t1[sl, :], sig[sl, 0:BLK],
                                                 gt[sl, :])
                            nc.vector.tensor_mul(t2[sl, :], sig[sl, BLK:2 * BLK],
                                                 st["c"][sl, :])
                            nc.vector.tensor_add(st["c"][sl, :], t1[sl, :],
                                                 t2[sl, :])

                        tch = tcp.tile([128, BLK], f32, tag="tc")
                        nc.scalar.activation(out=tch[sl, :], in_=st["c"][sl, :],
                                             func=mybir.ActivationFunctionType.Tanh)
                        # bf16 h feeds the next step's matmuls; a full-precision
                        # product is formed only when a block's output is due.
                        nc.vector.tensor_mul(st["hb"][sl, :],
                                             sig[sl, 2 * BLK:3 * BLK],
                                             tch[sl, :])
                        if need_f32_h:
                            hf = tmpp.tile([128, BLK], f32, tag="hf", name="hf")
                            nc.vector.tensor_mul(hf[sl, :],
                                                 sig[sl, 2 * BLK:3 * BLK],
                                                 tch[sl, :])
                    if not do_act:
                        need_f32_h = False

                    if need_f32_h:
                        if t == La - 1:
                            nc.sync.dma_start(out=out_d[grp["a"]],
                                              in_=hf[0:64, :])
                        if t == Lb - 1:
                            nc.sync.dma_start(out=out_d[grp["b"]],
                                              in_=hf[64:128, :])
                        for blk, half in cap_halves:
                            stg = ovstp.tile([64, BLK], f32, tag="ovst")
                            nc.vector.tensor_copy(stg, hf[half, :])
                            nc.sync.dma_start(
                                out=ov_d[blk["ov_idx"] * MAXL + t],
                                in_=stg[:])

    nc.compile()
    return nc


# --------------------------------------------------------------------------
# Entry point
# --------------------------------------------------------------------------

def kernel(emb, W_ih, W_hh, b_ih, b_hh, chars, lengths):
    from concourse.bass_utils import run_bass_kernel_spmd

    emb = np.asarray(emb, dtype=np.float32)
    W_ih = np.asarray(W_ih, dtype=np.float32)
    W_hh = np.asarray(W_hh, dtype=np.float32)
    b_ih = np.asarray(b_ih, dtype=np.float32)
    b_hh = np.asarray(b_hh, dtype=np.float32)
    chars = np.asarray(chars)
    lengths_np = np.asarray(lengths)

    n = chars.shape[0]

    # --- weight prep -------------------------------------------------------
    G = emb @ W_ih.T + b_ih + b_hh                      # [V, 4H]
    G = G[:, _GATE_PERM]
    gtab = np.zeros((128, GATE4), dtype=BF16)
    gtab[:V] = G.astype(BF16)
    whhT = W_hh.T[:, _GATE_PERM].astype(BF16)           # [H, 4H]
    zero = np.zeros_like(whhT)
    whhA = np.concatenate([whhT, zero], axis=0)         # [128, 4H]
    whhB = np.concatenate([zero, whhT], axis=0)         # [128, 4H]

    # --- word assignment ---------------------------------------------------
    plan = _plan(lengths_np)
    blocks, groups, sched = plan["blocks"], plan["groups"], plan["sched"]

    sig = (tuple((b["L"], b["is_ov"], b.get("cap_steps", ())) for b in blocks),
           tuple(sched))
    key = hash(sig)
    if key not in _PROGRAM_CACHE:
        _PROGRAM_CACHE[key] = _build_program(sig, blocks, groups, sched,
                                             plan["n_ov"])
    nc = _PROGRAM_CACHE[key]

    ohs = _build_onehots(plan, chars, lengths_np)
    in_maps = [{"oh": ohs[c], "gtab": gtab, "whha": whhA, "whhb": whhB}
               for c in range(NCORES)]

    res = run_bass_kernel_spmd(nc, in_maps, core_ids=list(range(NCORES)))
    kernel._last_nc = nc
    kernel._last_in_maps = in_maps

    # --- gather results ----------------------------------------------------
    outs = np.stack([r["out"] for r in res.results])    # [8, nb, H, BLK]
    ovs = np.stack([r["ov"] for r in res.results])      # [8, n_ov*16, H, BLK]

    result = np.empty((n, H), dtype=np.float32)
    for c in range(NCORES):
        for bi, blk in enumerate(blocks):
            words = plan["assign"][c][bi]
            valid = words >= 0
            if not valid.any():
                continue
            w = words[valid]
            cols = np.nonzero(valid)[0]
            if blk["is_ov"]:
                steps = lengths_np[w].astype(np.int64) - 1
                result[w] = ovs[c, blk["ov_idx"] * MAXL + steps, :, cols]
            else:
                result[w] = outs[c, bi, :, cols]
    return result

